# revision 1
# baseline (speedup 1.0000x reference)
"""v3: paired exp (one 1792-col ACT inst per q-block over a psum slot pair),
f16 PE transposes (Pool pre-casts f32->f16), DVE computes 2/16 k-tiles of exp
via 1-pass int16 Schraudolph, O lags S by 2 pairs (PT triple-buffered).

PSUM layout (f32 cols): slots 0..2 at 0/1024/2048 (S logits, rotation),
opsum bufs at 3072+{0,512} (129 cols each: 128 d + ones-column denominator).
Steady-state transposes write f16 tile slots at f32 cols 192..512 of the
opsum buf of parity (w+1)%2; bank-level PE-write/DVE-read exclusion is
enforced by gates: O(p) waits batch-(p+1) copies, window-w transposes wait
norm(w-3) and batch-(w-2) copies.
"""
import os
import numpy as np
import concourse.bass as bass
from concourse import mybir
from contextlib import ExitStack

F32 = mybir.dt.float32
F16 = mybir.dt.float16
I16 = mybir.dt.int16
I32 = mybir.dt.int32
EXP = mybir.ActivationFunctionType.Exp
SCALE = float(1.0 / np.sqrt(128.0))
LN2 = float(np.log(2.0))
A16 = (1 << 10) / LN2 * SCALE
B16 = 15.0 * (1 << 10) - 61.0
A32 = (1 << 23) / LN2 * SCALE
B32 = 127.0 * (1 << 23) - 500000.0

N_CORES = 8
LABELS = {}


def _lab(inst, label):
    try:
        LABELS[inst.ins.name] = label
    except Exception:
        pass
    return inst


def build_attention_nc(SEQ=2048, B=2, G=4):
    NO_DUMMY = bool(int(os.environ.get("BIS_NO_DUMMY", "0")))
    NO_SPLIT = bool(int(os.environ.get("BIS_NO_SPLIT", "0")))
    NO_EXPD = bool(int(os.environ.get("BIS_NO_EXPD", "0")))
    EXPD_DUMMY = bool(int(os.environ.get("BIS_EXPD_DUMMY", "0")))
    EXPD_N = int(os.environ.get("BIS_EXPD_N", "128"))
    EXPD_NOGATE = bool(int(os.environ.get("BIS_EXPD_NOGATE", "0")))
    def expd_on(p):
        return (not NO_EXPD) and p < EXPD_N
    ACT_W = 1024 if (NO_EXPD or EXPD_DUMMY) else 768
    D = 128
    T = SEQ // 128            # 16 k/q tiles per head
    H = B * G                 # 8 (b, g) heads per core
    NPH = T                   # pairs (q-blocks) per head
    NPAIR = H * NPH           # 128
    NW = NPAIR + 2            # windows (O lags 2)
    assert T == 16 and H == 8 and B == 2

    nc = bass.Bass()
    q_ext = nc.declare_dram_parameter("query", [SEQ, B, G, D], F32, isOutput=False)
    k_ext = nc.declare_dram_parameter("key", [SEQ, B, D], F32, isOutput=False)
    v_ext = nc.declare_dram_parameter("value", [SEQ, B, D], F32, isOutput=False)
    o_ext = nc.declare_dram_parameter("out", [SEQ, B, G, D], F32, isOutput=True)

    # loads in first-use order: K(b0), Q(h0..h3), K(b1), Q(h4..h7)
    loads = [("K", 0, None)] + [("Q", 0, g) for g in range(G)]
    loads += [("K", 1, None)] + [("Q", 1, g) for g in range(G)]
    NL = len(loads)           # 10

    def q_load_index(h):
        b, g = divmod(h, G)
        return b * (G + 1) + 1 + g

    # steady-state transpose windows per load (prologue covers l0 all + l1 t0)
    due_w = {
        2: list(range(6, 14)),      # Q h1
        3: list(range(18, 26)),     # Q h2
        4: list(range(34, 42)),     # Q h3
        5: list(range(40, 56)),     # K b1 (1/w)
        6: list(range(50, 58)),     # Q h4
        7: list(range(66, 74)),     # Q h5
        8: list(range(82, 90)),     # Q h6
        9: list(range(98, 106)),    # Q h7
    }
    per_w = {5: 1}

    trs_in_w = {w: [] for w in range(NW)}
    for i, t in enumerate(range(1, T)):       # l1 (Q h0) tiles 1..15
        trs_in_w[i // 2].append((1, t))
    for ld, ws in due_w.items():
        n = per_w.get(ld, 2)
        t = 0
        for w in ws:
            for _ in range(n):
                if t < T:
                    trs_in_w[w].append((ld, t))
                    t += 1
        assert t >= T, (ld, t)
    for w, trs in trs_in_w.items():
        assert len(trs) <= 5

    # cast (Pool f32->f16) events: (load, t0, t1, sem_load threshold)
    # (load, t0, t1, chunk): chunk 0/1 for split loads 0,1; else 0
    if NO_SPLIT:
        cast_events = [(ld, 0, 16, 0) for ld in range(NL)]
    else:
        cast_events = [(0, 0, 8, 0), (1, 0, 1, 0), (0, 8, 16, 1),
                       (1, 1, 8, 0), (1, 8, 16, 1)]
        cast_events += [(ld, 0, 16, 0) for ld in range(2, NL)]

    def cast_val_for(ld, t):
        for i, (l, t0, t1, _c) in enumerate(cast_events):
            if l == ld and t0 <= t < t1:
                return i + 1
        raise AssertionError((ld, t))

    # ---------------- schedule walk (mirrors emission exactly) ----------
    pe = 0
    pe_after_S = {}
    pe_after_O = {}
    pe_after_tr = {}
    load_last_pe = {}

    def note_tr(ld, t):
        nonlocal pe
        pe += 1
        pe_after_tr[(ld, t)] = pe
        load_last_pe[ld] = max(load_last_pe.get(ld, 0), pe)

    # prologue: l0 t0-7 -> bank6 s0-7; l1 t0 -> bank7 s0; S(0);
    # l0 t8-15 -> bank7 s1-7 + bank6 s0; S(1); then windows w>=0 trs/O.
    for t in range(8):
        note_tr(0, t)
    note_tr(1, 0)
    pe += 8
    pe_after_S[0] = pe
    for t in range(8, 16):
        note_tr(0, t)
    pe += 8
    pe_after_S[1] = pe
    for (ld, t) in trs_in_w.get(0, []):
        note_tr(ld, t)
    for w in range(NW):
        if w < NPAIR and w >= 1:
            pe += 8
            pe_after_S[2 * w] = pe
        if w >= 1:
            for (ld, t) in trs_in_w.get(w, []):
                note_tr(ld, t)
        if w >= 2:
            pe += 16
            pe_after_O[w - 2] = pe
        if w < NPAIR and w >= 1:
            pe += 8
            pe_after_S[2 * w + 1] = pe

    # DVE stream walk
    dve = 0
    copy_done = {}
    copy_batch_done = {}
    dve_exp_done = {}
    tsa_done = {}
    tsb_done = {}
    recips_done = {}
    mults_done = {}
    dve_ops = []

    def batch_runs(w):
        trs = trs_in_w.get(w, [])
        runs = []
        for (ld, t) in trs:
            if runs and runs[-1][0] == ld and runs[-1][2] == t:
                runs[-1] = [ld, runs[-1][1], t + 1]
            else:
                runs.append([ld, t, t + 1])
        return runs

    def note_copy_batch(w):
        nonlocal dve
        trs = trs_in_w.get(w, [])
        if not trs:
            copy_batch_done[w] = 0
            return
        runs = batch_runs(w)
        dve += len(runs)
        for (ld, t) in trs:
            copy_done[(ld, t)] = dve
        copy_batch_done[w] = dve
        dve_ops.append(("copies", w, runs))

    for kind, keys in [("b6", [(0, t) for t in range(8)]),
                       ("l1t0", [(1, 0)]),
                       ("b7a", [(0, t) for t in range(8, 15)]),
                       ("l0t15", [(0, 15)])]:
        dve += 1
        for kk in keys:
            copy_done[kk] = dve
        dve_ops.append(("pcopy", kind))

    for w in range(NW):
        if w < NPAIR and expd_on(w):
            dve += 1
            tsa_done[w] = dve
            dve_ops.append(("expts", w))
        note_copy_batch(w)
        if w >= 2:
            p = w - 2
            dve += 1
            recips_done[p] = dve
            dve_ops.append(("recip", p))
            dve += 1
            mults_done[p] = dve
            dve_ops.append(("mult", p))
        if w < NPAIR and expd_on(w):
            dve += 1
            tsb_done[w] = dve
            dve_exp_done[w] = dve
            dve_ops.append(("expcp", w))

    head_ready = {}
    for h in range(H):
        b = h // G
        js = [(q_load_index(h), t) for t in range(T)]
        js += [(b * (G + 1), t) for t in range(T)]
        head_ready[h] = max(copy_done[j] for j in js)

    # PT half mapping: halves ordered by ascending psum slot address
    pt_half = {}
    for p in range(NPAIR):
        if p % 3 == 1:     # slots (2,0) -> ascending (0,2): half0 = group 2p+1
            pt_half[2 * p] = 1
            pt_half[2 * p + 1] = 0
        else:
            pt_half[2 * p] = 0
            pt_half[2 * p + 1] = 1

    # ---------------- tensors ----------------
    ident = nc.alloc_sbuf_tensor("ident", [128, 128], F16)
    bias0 = nc.alloc_sbuf_tensor("bias0", [128, 1], F32)
    scr = nc.alloc_sbuf_tensor("scr", [128, 1], F32)
    scr32 = nc.alloc_sbuf_tensor("scr32", [128, 768], I32)
    scrPT = nc.alloc_sbuf_tensor("scrPT", [128, 512], F16)
    qnat = [nc.alloc_sbuf_tensor(f"qnat{i}", [128, T * 128], F32) for i in range(3)]
    qnat16 = [nc.alloc_sbuf_tensor(f"qnat16_{i}", [128, T * 128], F16)
              for i in range(3)]
    KT = [nc.alloc_sbuf_tensor(f"KT{b}", [128, T * 128], F16) for b in range(B)]
    QT = [nc.alloc_sbuf_tensor(f"QT{h}", [128, T * 128], F16) for h in range(H)]
    VT = [nc.alloc_sbuf_tensor(f"VT{b}", [128, T * 132], F16) for b in range(B)]
    PT = [nc.alloc_sbuf_tensor(f"PT{s}", [128, 2048], F16) for s in range(3)]
    rsb = [nc.alloc_sbuf_tensor(f"rsb{s}", [128, 1], F32) for s in range(2)]
    OS = [nc.alloc_sbuf_tensor(f"OS{s}", [128, T * 128], F32) for s in range(2)]
    psum = nc.alloc_psum_tensor("psum", [128, 4096], F32)

    pv = psum[:, :].rearrange("p (s c) -> p s c", c=1024)
    p16 = psum[:].bitcast(F16)           # [128, 8192]

    def spsum_mm(slot, ki):
        return psum[:, slot * 1024 + ki * 128: slot * 1024 + (ki + 1) * 128]

    def opsum(buf):
        off = 3072 + buf * 512
        return psum[:, off:off + 129]

    def exp_slots(p, c0, c1):
        r = p % 3
        if r == 0:
            return pv[:, 0:2, c0:c1]
        if r == 1:
            return pv[:, 0::2, c0:c1]
        return pv[:, 1:3, c0:c1]

    def tr_parity(w):
        return (w + 1) % 2

    def tr_psum_w(w, k):
        base = (6 + tr_parity(w)) * 1024 + 384 + k * 128
        return p16[:, base:base + 128]

    PRO_SLOT = {}       # (ld,t) -> f16 col base for prologue trs
    for t in range(8):
        PRO_SLOT[(0, t)] = 6 * 1024 + t * 128
    PRO_SLOT[(1, 0)] = 7 * 1024
    for t in range(8, 15):
        PRO_SLOT[(0, t)] = 7 * 1024 + (t - 7) * 128
    PRO_SLOT[(0, 15)] = 6 * 1024

    with ExitStack() as ctx:
        sem_pe = ctx.enter_context(nc.semaphore("sem_pe"))
        sem_act = ctx.enter_context(nc.semaphore("sem_act"))
        sem_dve = ctx.enter_context(nc.semaphore("sem_dve"))
        sem_cast = ctx.enter_context(nc.semaphore("sem_cast"))
        sem_pool = ctx.enter_context(nc.semaphore("sem_pool"))
        sem_load = {}
        for i in range(NL):
            nch = 1 if NO_SPLIT else (2 if i < 2 else 1)
            for c in range(nch):
                sem_load[(i, c)] = ctx.enter_context(
                    nc.semaphore(f"sem_load{i}_{c}"))
        sem_out = [ctx.enter_context(nc.semaphore(f"sem_out{h}"))
                   for h in range(H)]
        sem_v = [ctx.enter_context(nc.semaphore(f"sem_v{b}")) for b in range(B)]
        block = ctx.enter_context(nc.Block())

        @block.sync
        def _(sync):
            def ld_src(i):
                kind, b, g = loads[i]
                return k_ext[:, b, :] if kind == "K" else q_ext[:, b, g, :]

            def emit_load(i, t0, t1, chunk):
                src = ld_src(i).rearrange("(t p) d -> p t d", p=128)
                dst = qnat[i % 3][:].rearrange("p (t d) -> p t d", d=128)
                nc.sync.dma_start(
                    out=dst[:, t0:t1, :], in_=src[:, t0:t1, :],
                ).then_inc(sem_load[(i, chunk)], 16)

            if NO_SPLIT:
                emit_load(0, 0, 16, 0)
                emit_load(1, 0, 16, 0)
            else:
                emit_load(0, 0, 8, 0)
                emit_load(1, 0, 8, 0)
                emit_load(0, 8, 16, 1)
                emit_load(1, 8, 16, 1)
            for i in range(2, NL):
                if i >= 3:
                    nc.sync.wait_ge(sem_pe, load_last_pe[i - 3])
                emit_load(i, 0, 16, 0)
            for h in range(H):
                nc.sync.wait_ge(sem_out[h], 32)

        @block.gpsimd
        def _(gp):
            nc.gpsimd.memset(ident[:], 0.0).then_inc(sem_pool)
            nc.gpsimd.wait_ge(sem_pool, 1)
            nc.gpsimd.affine_select(
                out=ident[:], in_=ident[:],
                compare_op=mybir.AluOpType.not_equal, fill=1.0,
                base=0, pattern=[[-1, 128]], channel_multiplier=1,
            ).then_inc(sem_pool)
            nc.gpsimd.memset(bias0[:], 0.0).then_inc(sem_pool)
            for b in range(B):
                vt3 = VT[b][:].rearrange("p (t c) -> p t c", c=132)
                nc.gpsimd.memset(vt3[:, :, 128:129], 1.0).then_inc(sem_pool)
                nc.gpsimd.dma_start(
                    out=vt3[:, :, 0:128],
                    in_=v_ext[:, b, :].rearrange("(t p) d -> p t d", p=128),
                ).then_inc(sem_v[b], 16)

            def cast(idx):
                ld, t0, t1, chunk = cast_events[idx]
                nc.gpsimd.wait_ge(sem_load[(ld, chunk)], 16)
                nc.gpsimd.tensor_copy(
                    qnat16[ld % 3][:, t0 * 128:t1 * 128],
                    qnat[ld % 3][:, t0 * 128:t1 * 128],
                ).then_inc(sem_cast)

            def out_store(h, half):
                b, g = divmod(h, G)
                hf = T // 2
                oh = o_ext[:, b, g, :].rearrange("(t p) d -> p t d", p=128)
                osh = OS[h % 2][:].rearrange("p (t d) -> p t d", d=128)
                p_end = h * NPH + (half + 1) * hf - 1
                nc.gpsimd.wait_ge(sem_dve, mults_done[p_end])
                nc.gpsimd.dma_start(
                    out=oh[:, half * hf:(half + 1) * hf, :],
                    in_=osh[:, half * hf:(half + 1) * hf, :],
                ).then_inc(sem_out[h], 16)

            # c0..c4: l0/l1 chunks; c5..c12: loads 2..9
            if NO_SPLIT:
                order = ["c0", "c1", "c2", "c3", "c4", "o0a", "c5",
                         "o0b", "o1a", "c6", "o1b", "o2a", "c7", "o2b",
                         "o3a", "c8", "o3b", "c9", "o4a", "o4b",
                         "o5a", "o5b", "o6a", "o6b", "o7a", "o7b"]
            else:
                order = ["c0", "c1", "c2", "c3", "c4", "c5", "c6",
                         "c7", "o0a", "c8", "o0b", "o1a", "c9", "o1b",
                         "o2a", "c10", "o2b", "o3a", "c11", "o3b",
                         "c12", "o4a", "o4b", "o5a", "o5b",
                         "o6a", "o6b", "o7a", "o7b"]
            for op in order:
                if op[0] == "c":
                    cast(int(op[1:]))
                else:
                    out_store(int(op[1]), 0 if op[2] == "a" else 1)

        @block.tensor
        def _(te):
            nc.tensor.wait_ge(sem_pool, 2)
            seen_cast = set()
            last_dve_wait = [0]

            def twait(val):
                if val > last_dve_wait[0]:
                    last_dve_wait[0] = val
                    nc.tensor.wait_ge(sem_dve, val)

            def emit_tr(ld, t, dst):
                cv = cast_val_for(ld, t)
                if cv not in seen_cast:
                    seen_cast.add(cv)
                    nc.tensor.wait_ge(sem_cast, cv)
                _lab(nc.tensor.transpose(
                    dst, qnat16[ld % 3][:, t * 128:(t + 1) * 128], ident[:],
                ), f"tr(l{ld},t{t})").then_inc(sem_pe)

            def emit_S(g):
                p = g >> 1
                h = p // NPH
                slot = g % 3
                kp = g & 1
                b = h // G
                qc = p % NPH
                if g == 2 * h * NPH and h >= 1:
                    twait(head_ready[h])
                if h == 0 and qc > 0 and kp == 0:
                    twait(copy_done[(1, qc)])
                if g == 0:
                    twait(copy_done[(1, 0)])
                if g == 1:
                    twait(copy_done[(0, 15)])
                for ki in range(8):
                    kt = kp * 8 + ki
                    inst = nc.tensor.matmul(
                        spsum_mm(slot, ki),
                        KT[b][:, kt * 128:(kt + 1) * 128],
                        QT[h][:, qc * 128:(qc + 1) * 128],
                        start=True, stop=True, skip_group_check=True,
                    )
                    if ki == 0 and g >= 3:
                        inst._wait_ge(sem_act, g - 2)
                    _lab(inst, f"S(g{g},ki{ki})")
                    inst.then_inc(sem_pe)

            def emit_O(p):
                h = p // NPH
                b = h // G
                buf = p % 2
                if p == 0 or p == G * NPH:
                    nc.tensor.wait_ge(sem_v[b], 16)
                    nc.tensor.wait_ge(sem_pool, 4 + b)
                w_gate = 0
                if p >= 2:
                    w_gate = mults_done[p - 2]        # opsum buf reuse
                w_gate = max(w_gate, copy_batch_done.get(p + 1, 0))  # bank P10
                if expd_on(p):
                    w_gate = max(w_gate, tsb_done[p])
                if expd_on(p + 1):
                    w_gate = max(w_gate, tsa_done[p + 1])
                if w_gate:
                    twait(w_gate)
                vt3 = VT[b][:].rearrange("p (t c) -> p t c", c=132)
                kts = [0, 1, 2, 3, 4] + list(range(8, 16)) + [5, 6, 7]
                for i, kt in enumerate(kts):
                    g = 2 * p + (kt // 8)
                    half = pt_half[g]
                    ki = kt % 8
                    inst = nc.tensor.matmul(
                        opsum(buf),
                        PT[p % 3][:, half * 1024 + ki * 128:
                                  half * 1024 + (ki + 1) * 128],
                        vt3[:, kt, 0:129],
                        start=(i == 0), stop=(i == len(kts) - 1),
                        skip_group_check=True,
                    )
                    if i == 0:
                        inst._wait_ge(sem_act, 2 * p + 2)
                    _lab(inst, f"O(p{p},kt{kt})")
                    inst.then_inc(sem_pe)

            # prologue
            for t in range(8):
                emit_tr(0, t, p16[:, PRO_SLOT[(0, t)]:PRO_SLOT[(0, t)] + 128])
            emit_tr(1, 0, p16[:, PRO_SLOT[(1, 0)]:PRO_SLOT[(1, 0)] + 128])
            emit_S(0)
            for t in range(8, 16):
                if t == 15:
                    twait(copy_done[(0, 0)])   # bank6 s0 reuse
                emit_tr(0, t, p16[:, PRO_SLOT[(0, t)]:PRO_SLOT[(0, t)] + 128])
            emit_S(1)
            for k, (ld, t) in enumerate(trs_in_w.get(0, [])):
                emit_tr(ld, t, tr_psum_w(0, k))

            for w in range(NW):
                if w < NPAIR and w >= 1:
                    emit_S(2 * w)
                trs = trs_in_w.get(w, []) if w >= 1 else []
                if trs:
                    twait(copy_done[(0, 15)])          # all prologue copies
                    if w >= 3:
                        twait(mults_done[w - 3])       # norm done (same bank)
                    if w >= 2:
                        twait(copy_batch_done.get(w - 2, 0))  # slot reuse
                    for k, (ld, t) in enumerate(trs):
                        emit_tr(ld, t, tr_psum_w(w, k))
                if w >= 2:
                    emit_O(w - 2)
                if w < NPAIR and w >= 1:
                    emit_S(2 * w + 1)

        @block.scalar
        def _(sc):
            nc.scalar.wait_ge(sem_pool, 3)
            if not NO_DUMMY:
                nc.scalar.activation(                  # preload Exp table
                    out=scr[:, 0:1], in_=bias0[:, 0:1],
                    func=EXP, bias=bias0[:, 0:1], scale=1.0,
                )
            for p in range(NPAIR):
                ov = PT[p % 3][:, :].rearrange("p (s c) -> p s c", c=1024)
                for pos in range(2):
                    g = 2 * p + pos
                    slot = g % 3
                    half = pt_half[g]
                    aw = 640 if (expd_on(p) and pos == 0) else 1024
                    _lab(nc.scalar.activation(
                        out=ov[:, half:half + 1, 0:aw],
                        in_=pv[:, slot:slot + 1, 0:aw],
                        func=EXP, bias=bias0[:, 0:1], scale=SCALE,
                    )._wait_ge(sem_pe, pe_after_S[g]),
                        f"exp(g{g})").then_inc(sem_act)

        @block.vector
        def _(ve):
            def emit_pcopy(kind):
                if kind == "b6":
                    nc.vector.wait_ge(sem_pe, pe_after_tr[(0, 7)])
                    nc.vector.tensor_copy(
                        KT[0][:, 0:1024],
                        p16[:, 6 * 1024:6 * 1024 + 1024]).then_inc(sem_dve)
                elif kind == "l1t0":
                    nc.vector.wait_ge(sem_pe, pe_after_tr[(1, 0)])
                    nc.vector.tensor_copy(
                        QT[0][:, 0:128],
                        p16[:, 7 * 1024:7 * 1024 + 128]).then_inc(sem_dve)
                elif kind == "b7a":
                    nc.vector.wait_ge(sem_pe, pe_after_tr[(0, 14)])
                    nc.vector.tensor_copy(
                        KT[0][:, 1024:1920],
                        p16[:, 7 * 1024 + 128:8 * 1024]).then_inc(sem_dve)
                else:  # l0t15
                    nc.vector.wait_ge(sem_pe, pe_after_tr[(0, 15)])
                    nc.vector.tensor_copy(
                        KT[0][:, 1920:2048],
                        p16[:, 6 * 1024:6 * 1024 + 128]).then_inc(sem_dve)

            def emit_copies(w, runs):
                trs = trs_in_w[w]
                nc.vector.wait_ge(sem_pe, pe_after_tr[tuple(trs[-1])])
                k = 0
                base = (6 + tr_parity(w)) * 1024 + 384
                for (ld, t0, t1) in runs:
                    n = t1 - t0
                    src = p16[:, base + k * 128:base + (k + n) * 128]
                    kind, b, g = loads[ld]
                    tt = KT[b] if kind == "K" else QT[b * G + g]
                    _lab(nc.vector.tensor_copy(
                        tt[:, t0 * 128:t1 * 128], src), f"cp(w{w},l{ld},t{t0}-{t1})").then_inc(sem_dve)
                    k += n

            def emit_expts(p):
                g = 2 * p
                slot = g % 3
                nc.vector.wait_ge(sem_pe, pe_after_S[g])
                off = (p % 2) * 384
                _lab(nc.vector.tensor_scalar(
                    scr32[:, off:off + 384], pv[:, slot, 640:1024],
                    A32, B32, op0=mybir.AluOpType.mult,
                    op1=mybir.AluOpType.add,
                ), f"expDts({p})").then_inc(sem_dve)

            def emit_expcp(p):
                half = pt_half[2 * p]
                off = (p % 2) * 384
                nc.vector.wait_ge(sem_dve, tsa_done[p])   # scr32 RAW drain
                _lab(nc.vector.tensor_copy(
                    PT[p % 3][:, half * 1024 + 640:half * 1024 + 1024],
                    scr32[:, off:off + 384].bitcast(F32),
                ), f"expDcp({p})").then_inc(sem_dve)

            def emit_recip(p):
                buf = p % 2
                nc.vector.wait_ge(sem_pe, pe_after_O[p])
                if p >= 2:
                    nc.vector.wait_ge(sem_dve, mults_done[p - 2])
                _lab(nc.vector.reciprocal(
                    rsb[buf][:, 0:1], opsum(buf)[:, 128:129]), f"recip({p})").then_inc(sem_dve)

            def emit_mult(p):
                h = p // NPH
                qc = p % NPH
                buf = p % 2
                nc.vector.wait_ge(sem_dve, recips_done[p])
                if qc == 0 and h >= 2:
                    nc.vector.wait_ge(sem_out[h - 2], 32)
                _lab(nc.vector.tensor_scalar(
                    OS[h % 2][:, qc * 128:(qc + 1) * 128],
                    opsum(buf)[:, 0:128],
                    rsb[buf][:, 0:1],
                    None,
                    op0=mybir.AluOpType.mult,
                ), f"mult({p})").then_inc(sem_dve)

            n_pad = int(os.environ.get("BIS_DVE_PAD", "0"))
            pad_kind = os.environ.get("BIS_PAD_KIND", "tiny")
            for i in range(n_pad):
                if pad_kind == "tiny":
                    nc.vector.tensor_copy(scr[:, 0:1], bias0[:, 0:1])
                elif pad_kind == "ts_psum":
                    nc.vector.tensor_scalar(
                        scr32[:, :].rearrange("p (s c) -> p s c", c=256)[:, 0:2, :],
                        pv[:, 0::2, 768:1024],
                        A32, B32, op0=mybir.AluOpType.mult,
                        op1=mybir.AluOpType.add)
                elif pad_kind == "ts_sbuf":
                    nc.vector.tensor_scalar(
                        scr32[:, 0:256], scrPT[:, 0:256].bitcast(F32).to_broadcast([128, 256]) if False else scr32[:, 256:512].bitcast(F32),
                        A32, B32, op0=mybir.AluOpType.mult,
                        op1=mybir.AluOpType.add)
                elif pad_kind == "cp_big":
                    nc.vector.tensor_copy(
                        scrPT[:, :].rearrange("p (s c) -> p s c", c=256)[:, 0:2, :],
                        scr32[:].bitcast(F32).rearrange(
                            "p (s c) -> p s c", c=256)[:, 0:2, :])
            for op in dve_ops:
                if op[0] == "pcopy":
                    emit_pcopy(op[1])
                elif op[0] == "copies":
                    emit_copies(op[1], op[2])
                elif op[0] == "expts":
                    emit_expts(op[1])
                elif op[0] == "expcp":
                    emit_expcp(op[1])
                elif op[0] == "recip":
                    emit_recip(op[1])
                else:
                    emit_mult(op[1])

    return nc


_NC = None


def _get_nc():
    global _NC
    if _NC is None:
        _NC = build_attention_nc(2048, 2, 4)
    return _NC


def kernel(query, key, value):
    from concourse.bass_utils import run_bass_kernel_spmd

    query = np.ascontiguousarray(query, dtype=np.float32)
    key = np.ascontiguousarray(key, dtype=np.float32)
    value = np.ascontiguousarray(value, dtype=np.float32)
    G = query.shape[2] // key.shape[2]
    nc = _get_nc()
    in_maps = []
    for c in range(N_CORES):
        in_maps.append({
            "query": np.ascontiguousarray(query[:, :, c * G:(c + 1) * G, :]),
            "key": np.ascontiguousarray(key[:, :, c, :]),
            "value": np.ascontiguousarray(value[:, :, c, :]),
        })
    res = run_bass_kernel_spmd(nc, in_maps, list(range(N_CORES)))
    out = np.empty_like(query)
    for c in range(N_CORES):
        out[:, :, c * G:(c + 1) * G, :] = res.results[c]["out"]
    return out



# revision 16
# speedup vs baseline: 1.0132x; 1.0132x over previous
"""v4: GQA attention, 1 kv-head x 4 q-heads x B=2 per core, SEQ=2048.

Per-core dataflow (all dims per core):
  - Pool (gpsimd) casting-DMAs load Q/K f32 DRAM -> f16 SBUF (qnat16, 3-buf
    rotation over 10 loads) and V f32 -> f16 vt3 (128 d cols + ones col).
  - SP xbar DMA transposes qnat16 -> KT[b]/QT[h] (per 128x128 tile,
    out[:, t, :] = in[:, 128t:+128]^T). No PE transposes, no DVE copies.
  - PE: per q-block pair p (128 total): S = KT^T @ QT into psum slot (g%3,
    1024 f32 cols per group g=2p,2p+1), O = PT^T @ [V|1] into opsum
    (129 cols, banks 6/7), with warmup matmuls at t=0 to raise the p-state.
  - exp: ACT does cols 0:640 of each group (table Exp, scale=1/sqrt(128));
    DVE does cols 640:1024 via 1-pass int16 Schraudolph tensor_scalar
    (f32 psum in -> i16 out bitcast f16 in PT).
  - DVE: reciprocal of ones-column denom + scale O by it into OS (f32).
  - SP stores OS -> DRAM in quarter-head chunks.
"""
import numpy as np
import concourse.bass as bass
from concourse import mybir
from contextlib import ExitStack

F32 = mybir.dt.float32
F16 = mybir.dt.float16
I16 = mybir.dt.int16
EXP = mybir.ActivationFunctionType.Exp
SCALE = float(1.0 / np.sqrt(128.0))
LN2 = float(np.log(2.0))
A16 = (1 << 10) / LN2 * SCALE
B16 = 15.0 * (1 << 10) - 61.0

N_CORES = 8
N_WARM = 48
AW = 640                      # ACT exp cols per group; DVE does 1024-AW
LABELS = {}


def _lab(inst, label):
    try:
        LABELS[inst.ins.name] = label
    except Exception:
        pass
    return inst


def build_attention_nc(SEQ=2048, B=2, G=4):
    D = 128
    T = SEQ // 128            # 16 k/q tiles per head
    H = B * G                 # 8 (b, g) heads per core
    NPH = T                   # pairs (q-blocks) per head
    NPAIR = H * NPH           # 128
    NW = NPAIR + 2            # windows (O lags S by 2 pairs)
    assert T == 16 and H == 8 and B == 2

    nc = bass.Bass()
    q_ext = nc.declare_dram_parameter("query", [SEQ, B, G, D], F32, isOutput=False)
    k_ext = nc.declare_dram_parameter("key", [SEQ, B, D], F32, isOutput=False)
    v_ext = nc.declare_dram_parameter("value", [SEQ, B, D], F32, isOutput=False)
    o_ext = nc.declare_dram_parameter("out", [SEQ, B, G, D], F32, isOutput=True)

    # loads in first-use order: K(b0), Q(h0..h3), K(b1), Q(h4..h7)
    loads = [("K", 0, None)] + [("Q", 0, g) for g in range(G)]
    loads += [("K", 1, None)] + [("Q", 1, g) for g in range(G)]
    NL = len(loads)           # 10

    def q_load_index(h):
        b, g = divmod(h, G)
        return b * (G + 1) + 1 + g

    # load chunks: loads 0,1 split in half for startup pipelining
    chunks = []               # (load, t0, t1)
    chunks.append((0, 0, 8))
    chunks.append((1, 0, 8))
    chunks.append((0, 8, 16))
    chunks.append((1, 8, 16))
    for i in range(2, NL):
        chunks.append((i, 0, 16))
    NCH = len(chunks)

    # chunk indices holding each load's tiles (loads 0/1 split in two)
    load_chunks = {0: [0, 2], 1: [1, 3]}
    for i in range(2, NL):
        load_chunks[i] = [i + 2]

    def q_chunk(h, qc):
        if h == 0:
            return 1 if qc < 8 else 3
        return q_load_index(h) + 2

    def k_chunk(b, kp):
        if b == 0:
            return 0 if kp == 0 else 2
        return 7

    # ---------------- PE walk (sem_pe counts S/O matmuls only) ----------
    pe = 0
    pe_after_S = {}
    pe_after_O = {}
    pe += 8
    pe_after_S[0] = pe
    pe += 8
    pe_after_S[1] = pe
    for w in range(1, NW):
        if w < NPAIR:
            pe += 8
            pe_after_S[2 * w] = pe
        if w >= 2:
            pe += 16
            pe_after_O[w - 2] = pe
        if w < NPAIR:
            pe += 8
            pe_after_S[2 * w + 1] = pe

    # ---------------- DVE walk (sem_dve counter) ------------------------
    dve = 0
    dexp_done = {}
    recips_done = {}
    mults_done = {}
    dve_ops = []
    for w in range(NW):
        if w < NPAIR:
            dve += 1
            dexp_done[2 * w] = dve
            dve_ops.append(("dexp", 2 * w))
        if w >= 2:
            p = w - 2
            dve += 1
            recips_done[p] = dve
            dve_ops.append(("recip", p))
            dve += 1
            mults_done[p] = dve
            dve_ops.append(("mult", p))
        if w < NPAIR:
            dve += 1
            dexp_done[2 * w + 1] = dve
            dve_ops.append(("dexp", 2 * w + 1))

    # ---------------- tensors ----------------
    bias0 = nc.alloc_sbuf_tensor("bias0", [128, 1], F32)
    scr = nc.alloc_sbuf_tensor("scr", [128, 1], F32)
    warm = nc.alloc_sbuf_tensor("warm", [128, 128], F16)
    qnat16 = [nc.alloc_sbuf_tensor(f"qnat16_{i}", [128, T * 128], F16)
              for i in range(3)]
    KT = [nc.alloc_sbuf_tensor(f"KT{b}", [128, T * 128], F16) for b in range(B)]
    QT = [nc.alloc_sbuf_tensor(f"QT{h}", [128, T * 128], F16) for h in range(H)]
    VT = [nc.alloc_sbuf_tensor(f"VT{b}", [128, T * 132], F16) for b in range(B)]
    PT = [nc.alloc_sbuf_tensor(f"PT{s}", [128, 2048], F16) for s in range(3)]
    rsb = [nc.alloc_sbuf_tensor(f"rsb{s}", [128, 1], F32) for s in range(2)]
    OS = [nc.alloc_sbuf_tensor(f"OS{s}", [128, T * 128], F32) for s in range(2)]
    psum = nc.alloc_psum_tensor("psum", [128, 4096], F32)

    pv = psum[:, :].rearrange("p (s c) -> p s c", c=1024)
    PTI = [PT[s][:, :].bitcast(I16) for s in range(3)]

    def spsum_mm(slot, ki):
        return psum[:, slot * 1024 + ki * 128: slot * 1024 + (ki + 1) * 128]

    def opsum(buf):
        off = 3072 + buf * 512
        return psum[:, off:off + 129]

    # O matmul k-tile order: ACT-produced PT cols first, DVE-produced last
    O_KTS = [0, 1, 2, 3, 4, 8, 9, 10, 11, 12, 5, 6, 7, 13, 14, 15]
    DVE_WAIT_I = 10           # index in O_KTS needing dexp output

    with ExitStack() as ctx:
        sem_pe = ctx.enter_context(nc.semaphore("sem_pe"))
        sem_act = ctx.enter_context(nc.semaphore("sem_act"))
        sem_dve = ctx.enter_context(nc.semaphore("sem_dve"))
        sem_pool = ctx.enter_context(nc.semaphore("sem_pool"))
        sem_ld = [ctx.enter_context(nc.semaphore(f"sem_ld{c}"))
                  for c in range(NCH)]
        sem_tr = [ctx.enter_context(nc.semaphore(f"sem_tr{c}"))
                  for c in range(NCH)]
        sem_out = [ctx.enter_context(nc.semaphore(f"sem_out{h}"))
                   for h in range(H)]
        sem_v = [ctx.enter_context(nc.semaphore(f"sem_v{b}")) for b in range(B)]
        block = ctx.enter_context(nc.Block())

        def ld_src(i):
            kind, b, g = loads[i]
            return k_ext[:, b, :] if kind == "K" else q_ext[:, b, g, :]

        @block.gpsimd
        def _(gp):
            nc.gpsimd.memset(warm[:], 0.0).then_inc(sem_pool)
            nc.gpsimd.memset(bias0[:], 0.0).then_inc(sem_pool)
            for b in range(B):
                vt3 = VT[b][:].rearrange("p (t c) -> p t c", c=132)
                nc.gpsimd.memset(vt3[:, :, 128:129], 1.0).then_inc(sem_pool)

            def emit_load(ci):
                i, t0, t1 = chunks[ci]
                if i >= 3:
                    for cj in load_chunks[i - 3]:
                        nc.gpsimd.wait_ge(sem_tr[cj], 16)
                src = ld_src(i).rearrange("(t p) d -> p t d", p=128)
                dst = qnat16[i % 3][:].rearrange("p (t d) -> p t d", d=128)
                _lab(nc.gpsimd.dma_start(
                    out=dst[:, t0:t1, :], in_=src[:, t0:t1, :],
                ), f"L(c{ci},l{i})").then_inc(sem_ld[ci], 16)

            def emit_vload(b):
                vt3 = VT[b][:].rearrange("p (t c) -> p t c", c=132)
                _lab(nc.gpsimd.dma_start(
                    out=vt3[:, :, 0:128],
                    in_=v_ext[:, b, :].rearrange("(t p) d -> p t d", p=128),
                ), f"V({b})").then_inc(sem_v[b], 16)

            emit_load(0)          # K b0 t0-7
            emit_load(1)          # Q h0 t0-7
            emit_vload(0)
            emit_load(2)          # K b0 t8-15
            emit_load(3)          # Q h0 t8-15
            emit_load(4)          # load 2 (Q h1)
            emit_vload(1)
            for ci in range(5, NCH):
                emit_load(ci)

        @block.sync
        def _(sync):
            # xbar transposes, in chunk order
            for ci in range(NCH):
                i, t0, t1 = chunks[ci]
                nc.sync.wait_ge(sem_ld[ci], 16)
                kind, b, g = loads[i]
                tt = KT[b] if kind == "K" else QT[b * G + g]
                dst = tt[:].rearrange("p (t d) -> p t d", d=128)
                _lab(nc.sync.dma_start_transpose(
                    dst[:, t0:t1, :],
                    qnat16[i % 3][:, t0 * 128:t1 * 128],
                ), f"T(c{ci},l{i})").then_inc(sem_tr[ci], 16)

            # output stores, quarter-head granularity
            for h in range(H):
                b, g = divmod(h, G)
                oh = o_ext[:, b, g, :].rearrange("(t p) d -> p t d", p=128)
                osh = OS[h % 2][:].rearrange("p (t d) -> p t d", d=128)
                for q in range(4):
                    p_end = h * NPH + 4 * q + 3
                    nc.sync.wait_ge(sem_dve, mults_done[p_end])
                    _lab(nc.sync.dma_start(
                        out=oh[:, 4 * q:4 * q + 4, :],
                        in_=osh[:, 4 * q:4 * q + 4, :],
                    ), f"st(h{h},q{q})").then_inc(sem_out[h], 16)
            for h in range(H):
                nc.sync.wait_ge(sem_out[h], 64)

        @block.tensor
        def _(te):
            nc.tensor.wait_ge(sem_pool, 1)
            for i in range(N_WARM):
                _lab(nc.tensor.matmul(
                    psum[:, 3072:3200], warm[:], warm[:],
                    start=True, stop=True, skip_group_check=True,
                ), f"warm{i}")

            done_chunks = set()
            last_dve = [0]

            def chunk_wait(ci):
                if ci not in done_chunks:
                    done_chunks.add(ci)
                    nc.tensor.wait_ge(sem_tr[ci], 16)

            def dve_wait(val):
                if val > last_dve[0]:
                    last_dve[0] = val
                    nc.tensor.wait_ge(sem_dve, val)

            def emit_S(g):
                p = g >> 1
                h = p // NPH
                slot = g % 3
                kp = g & 1
                b = h // G
                qc = p % NPH
                chunk_wait(k_chunk(b, kp))
                chunk_wait(q_chunk(h, qc))
                if g >= 3:
                    dve_wait(dexp_done[g - 3])
                for ki in range(8):
                    kt = kp * 8 + ki
                    inst = nc.tensor.matmul(
                        spsum_mm(slot, ki),
                        KT[b][:, kt * 128:(kt + 1) * 128],
                        QT[h][:, qc * 128:(qc + 1) * 128],
                        start=True, stop=True, skip_group_check=True,
                    )
                    if ki == 0 and g >= 3:
                        inst._wait_ge(sem_act, g - 2)
                    _lab(inst, f"S(g{g},ki{ki})")
                    inst.then_inc(sem_pe)

            def emit_O(p):
                h = p // NPH
                b = h // G
                buf = p % 2
                if p == 0 or p == G * NPH:
                    nc.tensor.wait_ge(sem_v[b], 16)
                    nc.tensor.wait_ge(sem_pool, 3 + b)
                if p >= 2:
                    dve_wait(mults_done[p - 2])   # opsum buf reuse
                vt3 = VT[b][:].rearrange("p (t c) -> p t c", c=132)
                for i, kt in enumerate(O_KTS):
                    half = kt // 8
                    ki = kt % 8
                    inst = nc.tensor.matmul(
                        opsum(buf),
                        PT[p % 3][:, half * 1024 + ki * 128:
                                  half * 1024 + (ki + 1) * 128],
                        vt3[:, kt, 0:129],
                        start=(i == 0), stop=(i == len(O_KTS) - 1),
                        skip_group_check=True,
                    )
                    if i == 0:
                        inst._wait_ge(sem_act, 2 * p + 2)
                    if i == DVE_WAIT_I:
                        inst._wait_ge(sem_dve, dexp_done[2 * p + 1])
                    _lab(inst, f"O(p{p},kt{kt})")
                    inst.then_inc(sem_pe)

            emit_S(0)
            emit_S(1)
            for w in range(1, NW):
                if w < NPAIR:
                    emit_S(2 * w)
                if w >= 2:
                    emit_O(w - 2)
                if w < NPAIR:
                    emit_S(2 * w + 1)

        @block.scalar
        def _(sc):
            nc.scalar.wait_ge(sem_pool, 2)
            nc.scalar.activation(                  # preload Exp table
                out=scr[:, 0:1], in_=bias0[:, 0:1],
                func=EXP, bias=bias0[:, 0:1], scale=1.0,
            )
            for g in range(2 * NPAIR):
                p = g >> 1
                half = g & 1
                slot = g % 3
                ov = PT[p % 3][:, :].rearrange("p (s c) -> p s c", c=1024)
                _lab(nc.scalar.activation(
                    out=ov[:, half:half + 1, 0:AW],
                    in_=pv[:, slot:slot + 1, 0:AW],
                    func=EXP, bias=bias0[:, 0:1], scale=SCALE,
                )._wait_ge(sem_pe, pe_after_S[g]),
                    f"exp(g{g})").then_inc(sem_act)

        @block.vector
        def _(ve):
            def emit_dexp(g):
                p = g >> 1
                half = g & 1
                slot = g % 3
                nc.vector.wait_ge(sem_pe, pe_after_S[g])
                _lab(nc.vector.tensor_scalar(
                    PTI[p % 3][:, half * 1024 + AW:half * 1024 + 1024],
                    pv[:, slot, AW:1024],
                    A16, B16, op0=mybir.AluOpType.mult,
                    op1=mybir.AluOpType.add,
                ), f"dexp(g{g})").then_inc(sem_dve)

            def emit_recip(p):
                buf = p % 2
                nc.vector.wait_ge(sem_pe, pe_after_O[p])
                if p >= 2:
                    nc.vector.wait_ge(sem_dve, mults_done[p - 2])
                _lab(nc.vector.reciprocal(
                    rsb[buf][:, 0:1], opsum(buf)[:, 128:129]),
                    f"recip({p})").then_inc(sem_dve)

            def emit_mult(p):
                h = p // NPH
                qc = p % NPH
                buf = p % 2
                nc.vector.wait_ge(sem_dve, recips_done[p])
                if qc == 0 and h >= 2:
                    nc.vector.wait_ge(sem_out[h - 2], 64)
                _lab(nc.vector.tensor_scalar(
                    OS[h % 2][:, qc * 128:(qc + 1) * 128],
                    opsum(buf)[:, 0:128],
                    rsb[buf][:, 0:1],
                    None,
                    op0=mybir.AluOpType.mult,
                ), f"mult({p})").then_inc(sem_dve)

            for op in dve_ops:
                if op[0] == "dexp":
                    emit_dexp(op[1])
                elif op[0] == "recip":
                    emit_recip(op[1])
                else:
                    emit_mult(op[1])

    return nc


_NC = None


def _get_nc():
    global _NC
    if _NC is None:
        _NC = build_attention_nc(2048, 2, 4)
    return _NC


def kernel(query, key, value):
    from concourse.bass_utils import run_bass_kernel_spmd

    query = np.ascontiguousarray(query, dtype=np.float32)
    key = np.ascontiguousarray(key, dtype=np.float32)
    value = np.ascontiguousarray(value, dtype=np.float32)
    G = query.shape[2] // key.shape[2]
    nc = _get_nc()
    in_maps = []
    for c in range(N_CORES):
        in_maps.append({
            "query": np.ascontiguousarray(query[:, :, c * G:(c + 1) * G, :]),
            "key": np.ascontiguousarray(key[:, :, c, :]),
            "value": np.ascontiguousarray(value[:, :, c, :]),
        })
    res = run_bass_kernel_spmd(nc, in_maps, list(range(N_CORES)))
    out = np.empty_like(query)
    for c in range(N_CORES):
        out[:, :, c * G:(c + 1) * G, :] = res.results[c]["out"]
    return out


# revision 19
# speedup vs baseline: 1.0431x; 1.0294x over previous
"""v4: GQA attention, 1 kv-head x 4 q-heads x B=2 per core, SEQ=2048.

Per-core dataflow (all dims per core):
  - Pool (gpsimd) casting-DMAs load Q/K f32 DRAM -> f16 SBUF (qnat16, 3-buf
    rotation over 10 loads) and V f32 -> f16 vt3 (128 d cols + ones col).
  - SP xbar DMA transposes qnat16 -> KT[b]/QT[h] (per 128x128 tile,
    out[:, t, :] = in[:, 128t:+128]^T). No PE transposes, no DVE copies.
  - PE: per q-block pair p (128 total): S = KT^T @ QT into psum slot (g%3,
    1024 f32 cols per group g=2p,2p+1), O = PT^T @ [V|1] into opsum
    (129 cols, banks 6/7), with warmup matmuls at t=0 to raise the p-state.
  - exp: ACT does cols 0:640 of each group (table Exp, scale=1/sqrt(128));
    DVE does cols 640:1024 via 1-pass int16 Schraudolph tensor_scalar
    (f32 psum in -> i16 out bitcast f16 in PT).
  - DVE: reciprocal of ones-column denom + scale O by it into OS (f32).
  - SP stores OS -> DRAM in quarter-head chunks.
"""
import numpy as np
import concourse.bass as bass
from concourse import mybir
from contextlib import ExitStack

F32 = mybir.dt.float32
F16 = mybir.dt.float16
I16 = mybir.dt.int16
EXP = mybir.ActivationFunctionType.Exp
SCALE = float(1.0 / np.sqrt(128.0))
LN2 = float(np.log(2.0))
A16 = (1 << 10) / LN2 * SCALE
B16 = 15.0 * (1 << 10) - 61.0

N_CORES = 8
N_WARM = 100
AW = 640                      # ACT exp cols per group; DVE does 1024-AW
LABELS = {}


def _lab(inst, label):
    try:
        LABELS[inst.ins.name] = label
    except Exception:
        pass
    return inst


def build_attention_nc(SEQ=2048, B=2, G=4):
    D = 128
    T = SEQ // 128            # 16 k/q tiles per head
    H = B * G                 # 8 (b, g) heads per core
    NPH = T                   # pairs (q-blocks) per head
    NPAIR = H * NPH           # 128
    NW = NPAIR + 2            # windows (O lags S by 2 pairs)
    assert T == 16 and H == 8 and B == 2

    nc = bass.Bass()
    q_ext = nc.declare_dram_parameter("query", [SEQ, B, G, D], F32, isOutput=False)
    k_ext = nc.declare_dram_parameter("key", [SEQ, B, D], F32, isOutput=False)
    v_ext = nc.declare_dram_parameter("value", [SEQ, B, D], F32, isOutput=False)
    o_ext = nc.declare_dram_parameter("out", [SEQ, B, G, D], F32, isOutput=True)

    # loads in first-use order: K(b0), Q(h0..h3), K(b1), Q(h4..h7)
    loads = [("K", 0, None)] + [("Q", 0, g) for g in range(G)]
    loads += [("K", 1, None)] + [("Q", 1, g) for g in range(G)]
    NL = len(loads)           # 10

    def q_load_index(h):
        b, g = divmod(h, G)
        return b * (G + 1) + 1 + g

    # load chunks: loads 0,1 split in half for startup pipelining
    chunks = []               # (load, t0, t1)
    chunks.append((0, 0, 8))
    chunks.append((1, 0, 8))
    chunks.append((0, 8, 16))
    chunks.append((1, 8, 16))
    for i in range(2, NL):
        chunks.append((i, 0, 16))
    NCH = len(chunks)

    # chunk indices holding each load's tiles (loads 0/1 split in two)
    load_chunks = {0: [0, 2], 1: [1, 3]}
    for i in range(2, NL):
        load_chunks[i] = [i + 2]

    def q_chunk(h, qc):
        if h == 0:
            return 1 if qc < 8 else 3
        return q_load_index(h) + 2

    def k_chunk(b, kp):
        if b == 0:
            return 0 if kp == 0 else 2
        return 7

    # ---------------- PE walk (sem_pe counts S/O matmuls only) ----------
    pe = 0
    pe_after_S = {}
    pe_after_O = {}
    pe += 8
    pe_after_S[0] = pe
    pe += 8
    pe_after_S[1] = pe
    for w in range(1, NW):
        if w < NPAIR:
            pe += 8
            pe_after_S[2 * w] = pe
        if w >= 2:
            pe += 16
            pe_after_O[w - 2] = pe
        if w < NPAIR:
            pe += 8
            pe_after_S[2 * w + 1] = pe

    # ---------------- DVE walk (sem_dve counter) ------------------------
    dve = 0
    dexp_done = {}
    recips_done = {}
    mults_done = {}
    dve_ops = []
    for w in range(NW):
        if w < NPAIR:
            dve += 1
            dexp_done[2 * w] = dve
            dve_ops.append(("dexp", 2 * w))
        if w >= 2:
            p = w - 2
            dve += 1
            recips_done[p] = dve
            dve_ops.append(("recip", p))
            dve += 1
            mults_done[p] = dve
            dve_ops.append(("mult", p))
        if w < NPAIR:
            dve += 1
            dexp_done[2 * w + 1] = dve
            dve_ops.append(("dexp", 2 * w + 1))

    # ---------------- tensors ----------------
    bias0 = nc.alloc_sbuf_tensor("bias0", [128, 1], F32)
    scr = nc.alloc_sbuf_tensor("scr", [128, 1], F32)
    warm = nc.alloc_sbuf_tensor("warm", [128, 128], F16)
    qnat16 = [nc.alloc_sbuf_tensor(f"qnat16_{i}", [128, T * 128], F16)
              for i in range(3)]
    KT = [nc.alloc_sbuf_tensor(f"KT{b}", [128, T * 128], F16) for b in range(B)]
    QT = [nc.alloc_sbuf_tensor(f"QT{h}", [128, T * 128], F16) for h in range(H)]
    VT = [nc.alloc_sbuf_tensor(f"VT{b}", [128, T * 132], F16) for b in range(B)]
    PT = [nc.alloc_sbuf_tensor(f"PT{s}", [128, 2048], F16) for s in range(3)]
    rsb = [nc.alloc_sbuf_tensor(f"rsb{s}", [128, 1], F32) for s in range(2)]
    OS = [nc.alloc_sbuf_tensor(f"OS{s}", [128, T * 128], F32) for s in range(2)]
    psum = nc.alloc_psum_tensor("psum", [128, 4096], F32)

    pv = psum[:, :].rearrange("p (s c) -> p s c", c=1024)
    PTI = [PT[s][:, :].bitcast(I16) for s in range(3)]

    def spsum_mm(slot, ki):
        return psum[:, slot * 1024 + ki * 128: slot * 1024 + (ki + 1) * 128]

    def opsum(buf):
        off = 3072 + buf * 512
        return psum[:, off:off + 129]

    # O matmul k-tile order: ACT-produced PT cols first, DVE-produced last
    O_KTS = [0, 1, 2, 3, 4, 8, 9, 10, 11, 12, 5, 6, 7, 13, 14, 15]
    DVE_WAIT_I = 10           # index in O_KTS needing dexp output

    with ExitStack() as ctx:
        sem_pe = ctx.enter_context(nc.semaphore("sem_pe"))
        sem_act = ctx.enter_context(nc.semaphore("sem_act"))
        sem_dve = ctx.enter_context(nc.semaphore("sem_dve"))
        sem_pool = ctx.enter_context(nc.semaphore("sem_pool"))
        sem_ld = [ctx.enter_context(nc.semaphore(f"sem_ld{c}"))
                  for c in range(NCH)]
        sem_tr = [ctx.enter_context(nc.semaphore(f"sem_tr{c}"))
                  for c in range(NCH)]
        sem_out = [ctx.enter_context(nc.semaphore(f"sem_out{h}"))
                   for h in range(H)]
        sem_v = [ctx.enter_context(nc.semaphore(f"sem_v{b}")) for b in range(B)]
        block = ctx.enter_context(nc.Block())

        def ld_src(i):
            kind, b, g = loads[i]
            return k_ext[:, b, :] if kind == "K" else q_ext[:, b, g, :]

        @block.gpsimd
        def _(gp):
            def emit_memsets():
                nc.gpsimd.memset(bias0[:], 0.0).then_inc(sem_pool)
                for b in range(B):
                    vt3 = VT[b][:].rearrange("p (t c) -> p t c", c=132)
                    nc.gpsimd.memset(vt3[:, :, 128:129], 1.0).then_inc(sem_pool)

            def emit_load(ci):
                i, t0, t1 = chunks[ci]
                if i >= 3:
                    for cj in load_chunks[i - 3]:
                        nc.gpsimd.wait_ge(sem_tr[cj], 16)
                src = ld_src(i).rearrange("(t p) d -> p t d", p=128)
                dst = qnat16[i % 3][:].rearrange("p (t d) -> p t d", d=128)
                _lab(nc.gpsimd.dma_start(
                    out=dst[:, t0:t1, :], in_=src[:, t0:t1, :],
                ), f"L(c{ci},l{i})").then_inc(sem_ld[ci], 16)

            def emit_vload(b):
                vt3 = VT[b][:].rearrange("p (t c) -> p t c", c=132)
                _lab(nc.gpsimd.dma_start(
                    out=vt3[:, :, 0:128],
                    in_=v_ext[:, b, :].rearrange("(t p) d -> p t d", p=128),
                ), f"V({b})").then_inc(sem_v[b], 16)

            nc.gpsimd.memset(warm[:], 0.0).then_inc(sem_pool)
            emit_load(0)          # K b0 t0-7
            emit_load(1)          # Q h0 t0-7
            emit_load(2)          # K b0 t8-15
            emit_load(3)          # Q h0 t8-15
            emit_memsets()
            emit_vload(0)
            emit_load(4)          # load 2 (Q h1)
            emit_vload(1)
            for ci in range(5, NCH):
                emit_load(ci)

        @block.sync
        def _(sync):
            # xbar transposes, in chunk order
            for ci in range(NCH):
                i, t0, t1 = chunks[ci]
                nc.sync.wait_ge(sem_ld[ci], 16)
                kind, b, g = loads[i]
                tt = KT[b] if kind == "K" else QT[b * G + g]
                dst = tt[:].rearrange("p (t d) -> p t d", d=128)
                _lab(nc.sync.dma_start_transpose(
                    dst[:, t0:t1, :],
                    qnat16[i % 3][:, t0 * 128:t1 * 128],
                ), f"T(c{ci},l{i})").then_inc(sem_tr[ci], 16)

            # output stores, quarter-head granularity
            for h in range(H):
                b, g = divmod(h, G)
                oh = o_ext[:, b, g, :].rearrange("(t p) d -> p t d", p=128)
                osh = OS[h % 2][:].rearrange("p (t d) -> p t d", d=128)
                for q in range(4):
                    p_end = h * NPH + 4 * q + 3
                    nc.sync.wait_ge(sem_dve, mults_done[p_end])
                    _lab(nc.sync.dma_start(
                        out=oh[:, 4 * q:4 * q + 4, :],
                        in_=osh[:, 4 * q:4 * q + 4, :],
                    ), f"st(h{h},q{q})").then_inc(sem_out[h], 16)
            for h in range(H):
                nc.sync.wait_ge(sem_out[h], 64)

        @block.tensor
        def _(te):
            nc.tensor.wait_ge(sem_pool, 1)
            for i in range(N_WARM):
                _lab(nc.tensor.matmul(
                    psum[:, 3072:3200], warm[:], warm[:],
                    start=True, stop=True, skip_group_check=True,
                ), f"warm{i}")

            done_chunks = set()
            last_dve = [0]

            def chunk_wait(ci):
                if ci not in done_chunks:
                    done_chunks.add(ci)
                    nc.tensor.wait_ge(sem_tr[ci], 16)

            def dve_wait(val):
                if val > last_dve[0]:
                    last_dve[0] = val
                    nc.tensor.wait_ge(sem_dve, val)

            def emit_S(g):
                p = g >> 1
                h = p // NPH
                slot = g % 3
                kp = g & 1
                b = h // G
                qc = p % NPH
                chunk_wait(k_chunk(b, kp))
                chunk_wait(q_chunk(h, qc))
                if g >= 3:
                    dve_wait(dexp_done[g - 3])
                for ki in range(8):
                    kt = kp * 8 + ki
                    inst = nc.tensor.matmul(
                        spsum_mm(slot, ki),
                        KT[b][:, kt * 128:(kt + 1) * 128],
                        QT[h][:, qc * 128:(qc + 1) * 128],
                        start=True, stop=True, skip_group_check=True,
                    )
                    if ki == 0 and g >= 3:
                        inst._wait_ge(sem_act, g - 2)
                    _lab(inst, f"S(g{g},ki{ki})")
                    inst.then_inc(sem_pe)

            def emit_O(p):
                h = p // NPH
                b = h // G
                buf = p % 2
                if p == 0 or p == G * NPH:
                    nc.tensor.wait_ge(sem_v[b], 16)
                    nc.tensor.wait_ge(sem_pool, 3 + b)
                if p >= 2:
                    dve_wait(mults_done[p - 2])   # opsum buf reuse
                vt3 = VT[b][:].rearrange("p (t c) -> p t c", c=132)
                for i, kt in enumerate(O_KTS):
                    half = kt // 8
                    ki = kt % 8
                    inst = nc.tensor.matmul(
                        opsum(buf),
                        PT[p % 3][:, half * 1024 + ki * 128:
                                  half * 1024 + (ki + 1) * 128],
                        vt3[:, kt, 0:129],
                        start=(i == 0), stop=(i == len(O_KTS) - 1),
                        skip_group_check=True,
                    )
                    if i == 0:
                        inst._wait_ge(sem_act, 2 * p + 2)
                    if i == DVE_WAIT_I:
                        inst._wait_ge(sem_dve, dexp_done[2 * p + 1])
                    _lab(inst, f"O(p{p},kt{kt})")
                    inst.then_inc(sem_pe)

            emit_S(0)
            emit_S(1)
            for w in range(1, NW):
                if w < NPAIR:
                    emit_S(2 * w)
                if w >= 2:
                    emit_O(w - 2)
                if w < NPAIR:
                    emit_S(2 * w + 1)

        @block.scalar
        def _(sc):
            nc.scalar.wait_ge(sem_pool, 2)
            nc.scalar.activation(                  # preload Exp table
                out=scr[:, 0:1], in_=bias0[:, 0:1],
                func=EXP, bias=bias0[:, 0:1], scale=1.0,
            )
            for g in range(2 * NPAIR):
                p = g >> 1
                half = g & 1
                slot = g % 3
                ov = PT[p % 3][:, :].rearrange("p (s c) -> p s c", c=1024)
                _lab(nc.scalar.activation(
                    out=ov[:, half:half + 1, 0:AW],
                    in_=pv[:, slot:slot + 1, 0:AW],
                    func=EXP, bias=bias0[:, 0:1], scale=SCALE,
                )._wait_ge(sem_pe, pe_after_S[g]),
                    f"exp(g{g})").then_inc(sem_act)

        @block.vector
        def _(ve):
            def emit_dexp(g):
                p = g >> 1
                half = g & 1
                slot = g % 3
                nc.vector.wait_ge(sem_pe, pe_after_S[g])
                _lab(nc.vector.tensor_scalar(
                    PTI[p % 3][:, half * 1024 + AW:half * 1024 + 1024],
                    pv[:, slot, AW:1024],
                    A16, B16, op0=mybir.AluOpType.mult,
                    op1=mybir.AluOpType.add,
                ), f"dexp(g{g})").then_inc(sem_dve)

            def emit_recip(p):
                buf = p % 2
                nc.vector.wait_ge(sem_pe, pe_after_O[p])
                if p >= 2:
                    nc.vector.wait_ge(sem_dve, mults_done[p - 2])
                _lab(nc.vector.reciprocal(
                    rsb[buf][:, 0:1], opsum(buf)[:, 128:129]),
                    f"recip({p})").then_inc(sem_dve)

            def emit_mult(p):
                h = p // NPH
                qc = p % NPH
                buf = p % 2
                nc.vector.wait_ge(sem_dve, recips_done[p])
                if qc == 0 and h >= 2:
                    nc.vector.wait_ge(sem_out[h - 2], 64)
                _lab(nc.vector.tensor_scalar(
                    OS[h % 2][:, qc * 128:(qc + 1) * 128],
                    opsum(buf)[:, 0:128],
                    rsb[buf][:, 0:1],
                    None,
                    op0=mybir.AluOpType.mult,
                ), f"mult({p})").then_inc(sem_dve)

            for op in dve_ops:
                if op[0] == "dexp":
                    emit_dexp(op[1])
                elif op[0] == "recip":
                    emit_recip(op[1])
                else:
                    emit_mult(op[1])

    return nc


_NC = None


def _get_nc():
    global _NC
    if _NC is None:
        _NC = build_attention_nc(2048, 2, 4)
    return _NC


def kernel(query, key, value):
    from concourse.bass_utils import run_bass_kernel_spmd

    query = np.ascontiguousarray(query, dtype=np.float32)
    key = np.ascontiguousarray(key, dtype=np.float32)
    value = np.ascontiguousarray(value, dtype=np.float32)
    G = query.shape[2] // key.shape[2]
    nc = _get_nc()
    in_maps = []
    for c in range(N_CORES):
        in_maps.append({
            "query": np.ascontiguousarray(query[:, :, c * G:(c + 1) * G, :]),
            "key": np.ascontiguousarray(key[:, :, c, :]),
            "value": np.ascontiguousarray(value[:, :, c, :]),
        })
    res = run_bass_kernel_spmd(nc, in_maps, list(range(N_CORES)))
    out = np.empty_like(query)
    for c in range(N_CORES):
        out[:, :, c * G:(c + 1) * G, :] = res.results[c]["out"]
    return out


# revision 21
# speedup vs baseline: 1.0767x; 1.0322x over previous
"""v5: GQA attention, 1 kv-head x 4 q-heads x B=2 per core, SEQ=2048.

Per-core dataflow:
  - Startup fast path: K b0 (both halves) + Q h0 lo-half loaded as f32 via SP
    HWDGE into qf32 staging, PE f32-transposes into free psum (slot0/1 +
    banks 6/7), DVE cast-copies psum f32 -> KT0/QT0 f16. Avoids the Pool
    SWDGE prep + xbar chain on the critical path to S(0)/S(1).
  - All other loads: Pool (gpsimd) casting-DMAs f32 DRAM -> f16 qnat16
    (3-buf rotation), then SP xbar DMA transposes -> KT/QT
    (out[:, t, :] = in[:, 128t:+128]^T per 128x128 tile).
  - V: Pool casting-DMA f32 -> f16 vt3 (128 d cols + ones col).
  - PE: per q-block pair p (128): S = KT^T @ QT into psum slot g%3 (1024
    f32 cols per group g=2p,2p+1), O = PT^T @ [V|1] into opsum (129 cols,
    banks 6/7); warmup matmuls interleaved at start to hold the p-state.
  - exp: ACT does cols 0:640 of each group; DVE does cols 640:1024 via
    1-pass int16 Schraudolph tensor_scalar (f32 psum -> i16 bitcast f16 PT).
  - DVE: reciprocal of ones-col denominator, scale O into OS (f32).
  - SP stores OS -> DRAM in quarter-head chunks (last quarter split in two).
"""
import numpy as np
import concourse.bass as bass
from concourse import mybir
from contextlib import ExitStack

F32 = mybir.dt.float32
F16 = mybir.dt.float16
I16 = mybir.dt.int16
EXP = mybir.ActivationFunctionType.Exp
SCALE = float(1.0 / np.sqrt(128.0))
LN2 = float(np.log(2.0))
A16 = (1 << 10) / LN2 * SCALE
B16 = 15.0 * (1 << 10) - 61.0

N_CORES = 8
N_WARM = 40                   # warmups before first PE transpose
N_WARM2 = 11                  # fill between tr groups
AW = 640                      # ACT exp cols per group; DVE does 1024-AW
LABELS = {}


def _lab(inst, label):
    try:
        LABELS[inst.ins.name] = label
    except Exception:
        pass
    return inst


def build_attention_nc(SEQ=2048, B=2, G=4):
    D = 128
    T = SEQ // 128            # 16 k/q tiles per head
    H = B * G                 # 8 (b, g) heads per core
    NPH = T                   # pairs (q-blocks) per head
    NPAIR = H * NPH           # 128
    NW = NPAIR + 2            # windows (O lags S by 2 pairs)
    assert T == 16 and H == 8 and B == 2

    nc = bass.Bass()
    q_ext = nc.declare_dram_parameter("query", [SEQ, B, G, D], F32, isOutput=False)
    k_ext = nc.declare_dram_parameter("key", [SEQ, B, D], F32, isOutput=False)
    v_ext = nc.declare_dram_parameter("value", [SEQ, B, D], F32, isOutput=False)
    o_ext = nc.declare_dram_parameter("out", [SEQ, B, G, D], F32, isOutput=True)

    # loads in first-use order: K(b0), Q(h0..h3), K(b1), Q(h4..h7)
    loads = [("K", 0, None)] + [("Q", 0, g) for g in range(G)]
    loads += [("K", 1, None)] + [("Q", 1, g) for g in range(G)]
    NL = len(loads)           # 10

    def q_load_index(h):
        b, g = divmod(h, G)
        return b * (G + 1) + 1 + g

    # fast-path f32 SP loads: (dst qf32 idx, load, t0, t1)
    FAST = [(0, 0, 0, 8), (1, 1, 0, 8), (2, 0, 8, 16)]
    # Pool/xbar chunks: Q h0 hi-half, then loads 2..9 whole
    pool_chunks = [(1, 8, 16)] + [(i, 0, 16) for i in range(2, NL)]
    NPC = len(pool_chunks)    # 9

    def pc_of_load(i):        # xbar chunk index of load i (i >= 2)
        return i - 1

    def q_pc(h, qc):          # xbar chunk for S's Q tile; None if fast path
        if h == 0:
            return 0 if qc >= 8 else None
        return pc_of_load(q_load_index(h))

    def k_pc(b):              # xbar chunk for K; None if fast path (b0)
        return None if b == 0 else pc_of_load(5)

    # ---------------- PE walk (sem_pe: f32 trs + S/O matmuls) -----------
    pe = 24                   # 3 groups of 8 f32 transposes
    pe_after_S = {}
    pe_after_O = {}
    pe += 8
    pe_after_S[0] = pe
    pe += 8
    pe_after_S[1] = pe
    for w in range(1, NW):
        if w < NPAIR:
            pe += 8
            pe_after_S[2 * w] = pe
        if w >= 2:
            pe += 16
            pe_after_O[w - 2] = pe
        if w < NPAIR:
            pe += 8
            pe_after_S[2 * w + 1] = pe

    # ---------------- DVE walk (sem_dve counter) ------------------------
    dve = 3                   # cpK0, cpQ0, cpK1 prepended
    CP_K0, CP_Q0, CP_K1 = 1, 2, 3
    dexp_done = {}
    recips_done = {}
    mults_done = {}
    dve_ops = []
    for w in range(NW):
        if w < NPAIR:
            dve += 1
            dexp_done[2 * w] = dve
            dve_ops.append(("dexp", 2 * w))
        if w >= 2:
            p = w - 2
            dve += 1
            recips_done[p] = dve
            dve_ops.append(("recip", p))
            dve += 1
            mults_done[p] = dve
            dve_ops.append(("mult", p))
        if w < NPAIR:
            dve += 1
            dexp_done[2 * w + 1] = dve
            dve_ops.append(("dexp", 2 * w + 1))

    # ---------------- tensors ----------------
    bias0 = nc.alloc_sbuf_tensor("bias0", [128, 1], F32)
    scr = nc.alloc_sbuf_tensor("scr", [128, 1], F32)
    warm = nc.alloc_sbuf_tensor("warm", [128, 128], F16)
    ident = nc.alloc_sbuf_tensor("ident", [128, 128], F32)
    qf32 = [nc.alloc_sbuf_tensor(f"qf32_{j}", [128, 1024], F32)
            for j in range(3)]
    qnat16 = [nc.alloc_sbuf_tensor(f"qnat16_{i}", [128, T * 128], F16)
              for i in range(3)]
    KT = [nc.alloc_sbuf_tensor(f"KT{b}", [128, T * 128], F16) for b in range(B)]
    QT = [nc.alloc_sbuf_tensor(f"QT{h}", [128, T * 128], F16) for h in range(H)]
    VT = [nc.alloc_sbuf_tensor(f"VT{b}", [128, T * 132], F16) for b in range(B)]
    PT = [nc.alloc_sbuf_tensor(f"PT{s}", [128, 2048], F16) for s in range(3)]
    rsb = [nc.alloc_sbuf_tensor(f"rsb{s}", [128, 1], F32) for s in range(2)]
    OS = [nc.alloc_sbuf_tensor(f"OS{s}", [128, T * 128], F32) for s in range(2)]
    psum = nc.alloc_psum_tensor("psum", [128, 4096], F32)

    pv = psum[:, :].rearrange("p (s c) -> p s c", c=1024)
    PTI = [PT[s][:, :].bitcast(I16) for s in range(3)]

    def spsum_mm(slot, ki):
        return psum[:, slot * 1024 + ki * 128: slot * 1024 + (ki + 1) * 128]

    def opsum(buf):
        off = 3072 + buf * 512
        return psum[:, off:off + 129]

    # f32 transpose staging areas (psum col base per fast-path group)
    TR_BASE = {0: 3072, 1: 0, 2: 1024}   # K-lo: banks 6/7, Q-lo: slot0, K-hi: slot1

    # O matmul k-tile order: ACT-produced PT cols first, DVE-produced last
    O_KTS = [0, 1, 2, 3, 4, 8, 9, 10, 11, 12, 5, 6, 7, 13, 14, 15]
    DVE_WAIT_I = 10           # index in O_KTS needing dexp output
    ODD_ACT_I = 5             # first index reading the odd group's PT half

    with ExitStack() as ctx:
        sem_pe = ctx.enter_context(nc.semaphore("sem_pe"))
        sem_act = ctx.enter_context(nc.semaphore("sem_act"))
        sem_dve = ctx.enter_context(nc.semaphore("sem_dve"))
        sem_pool = ctx.enter_context(nc.semaphore("sem_pool"))
        sem_ldf = [ctx.enter_context(nc.semaphore(f"sem_ldf{j}"))
                   for j in range(3)]
        sem_ld = [ctx.enter_context(nc.semaphore(f"sem_ld{c}"))
                  for c in range(NPC)]
        sem_tr = [ctx.enter_context(nc.semaphore(f"sem_tr{c}"))
                  for c in range(NPC)]
        sem_out = [ctx.enter_context(nc.semaphore(f"sem_out{h}"))
                   for h in range(H)]
        sem_v = [ctx.enter_context(nc.semaphore(f"sem_v{b}")) for b in range(B)]
        block = ctx.enter_context(nc.Block())

        def ld_src(i):
            kind, b, g = loads[i]
            return k_ext[:, b, :] if kind == "K" else q_ext[:, b, g, :]

        @block.gpsimd
        def _(gp):
            # sem_pool: 1 ident0, 2 ident, 3 warm, 4 bias0, 5 vt0, 6 vt1
            nc.gpsimd.memset(ident[:], 0.0).then_inc(sem_pool)
            nc.gpsimd.wait_ge(sem_pool, 1)
            nc.gpsimd.affine_select(
                out=ident[:], in_=ident[:],
                compare_op=mybir.AluOpType.not_equal, fill=1.0,
                base=0, pattern=[[-1, 128]], channel_multiplier=1,
            ).then_inc(sem_pool)
            nc.gpsimd.memset(warm[:], 0.0).then_inc(sem_pool)

            def emit_load(pc):
                i, t0, t1 = pool_chunks[pc]
                if i >= 4:
                    nc.gpsimd.wait_ge(sem_tr[pc_of_load(i - 3)], 16)
                src = ld_src(i).rearrange("(t p) d -> p t d", p=128)
                dst = qnat16[i % 3][:].rearrange("p (t d) -> p t d", d=128)
                _lab(nc.gpsimd.dma_start(
                    out=dst[:, t0:t1, :], in_=src[:, t0:t1, :],
                ), f"L(pc{pc},l{i})").then_inc(sem_ld[pc], 16)

            def emit_vload(b):
                vt3 = VT[b][:].rearrange("p (t c) -> p t c", c=132)
                _lab(nc.gpsimd.dma_start(
                    out=vt3[:, :, 0:128],
                    in_=v_ext[:, b, :].rearrange("(t p) d -> p t d", p=128),
                ), f"V({b})").then_inc(sem_v[b], 16)

            emit_load(0)          # Q h0 t8-15
            emit_vload(0)
            nc.gpsimd.memset(bias0[:], 0.0).then_inc(sem_pool)
            for b in range(B):
                vt3 = VT[b][:].rearrange("p (t c) -> p t c", c=132)
                nc.gpsimd.memset(vt3[:, :, 128:129], 1.0).then_inc(sem_pool)
            emit_load(1)          # Q h1
            emit_vload(1)
            for pc in range(2, NPC):
                emit_load(pc)

        @block.sync
        def _(sync):
            # fast-path f32 loads
            for j, i, t0, t1 in FAST:
                src = ld_src(i).rearrange("(t p) d -> p t d", p=128)
                dst = qf32[j][:].rearrange("p (t d) -> p t d", d=128)
                _lab(nc.sync.dma_start(
                    out=dst[:, 0:8, :], in_=src[:, t0:t1, :],
                ), f"Lf{j}").then_inc(sem_ldf[j], 16)

            # xbar transposes for pool chunks
            for pc in range(NPC):
                i, t0, t1 = pool_chunks[pc]
                nc.sync.wait_ge(sem_ld[pc], 16)
                kind, b, g = loads[i]
                tt = KT[b] if kind == "K" else QT[b * G + g]
                dst = tt[:].rearrange("p (t d) -> p t d", d=128)
                _lab(nc.sync.dma_start_transpose(
                    dst[:, t0:t1, :],
                    qnat16[i % 3][:, t0 * 128:t1 * 128],
                ), f"T(pc{pc},l{i})").then_inc(sem_tr[pc], 16)

            # output stores, quarter-head granularity (last quarter split)
            def store(h, t0, t1, p_end):
                b, g = divmod(h, G)
                oh = o_ext[:, b, g, :].rearrange("(t p) d -> p t d", p=128)
                osh = OS[h % 2][:].rearrange("p (t d) -> p t d", d=128)
                nc.sync.wait_ge(sem_dve, mults_done[p_end])
                _lab(nc.sync.dma_start(
                    out=oh[:, t0:t1, :], in_=osh[:, t0:t1, :],
                ), f"st(h{h},t{t0})").then_inc(sem_out[h], 16)

            for h in range(H):
                for q in range(4):
                    if h == H - 1 and q == 3:
                        store(h, 12, 14, h * NPH + 13)
                        store(h, 14, 16, h * NPH + 15)
                    else:
                        store(h, 4 * q, 4 * q + 4, h * NPH + 4 * q + 3)
            for h in range(H):
                nc.sync.wait_ge(sem_out[h], 80 if h == H - 1 else 64)

        @block.tensor
        def _(te):
            nc.tensor.wait_ge(sem_pool, 3)
            nwarm = [0]

            def emit_warm(n):
                for _ in range(n):
                    nwarm[0] += 1
                    _lab(nc.tensor.matmul(
                        psum[:, 2048:2176], warm[:], warm[:],
                        start=True, stop=True, skip_group_check=True,
                    ), f"warm{nwarm[0]}")

            def emit_ftr(j):
                nc.tensor.wait_ge(sem_ldf[j], 16)
                base = TR_BASE[j]
                for t in range(8):
                    _lab(nc.tensor.transpose(
                        psum[:, base + t * 128: base + (t + 1) * 128],
                        qf32[j][:, t * 128:(t + 1) * 128], ident[:],
                    ), f"ftr(j{j},t{t})").then_inc(sem_pe)

            done_pc = set()
            last_dve = [0]

            def pc_wait(pc):
                if pc is not None and pc not in done_pc:
                    done_pc.add(pc)
                    nc.tensor.wait_ge(sem_tr[pc], 16)

            def dve_wait(val):
                if val > last_dve[0]:
                    last_dve[0] = val
                    nc.tensor.wait_ge(sem_dve, val)

            def emit_S(g):
                p = g >> 1
                h = p // NPH
                slot = g % 3
                kp = g & 1
                b = h // G
                qc = p % NPH
                pc_wait(k_pc(b))
                pc_wait(q_pc(h, qc))
                if g == 0:
                    dve_wait(CP_Q0)          # covers cpK0 too
                elif g == 1:
                    dve_wait(CP_K1)
                if g >= 3:
                    dve_wait(dexp_done[g - 3])
                for ki in range(8):
                    kt = kp * 8 + ki
                    inst = nc.tensor.matmul(
                        spsum_mm(slot, ki),
                        KT[b][:, kt * 128:(kt + 1) * 128],
                        QT[h][:, qc * 128:(qc + 1) * 128],
                        start=True, stop=True, skip_group_check=True,
                    )
                    if ki == 0 and g >= 3:
                        inst._wait_ge(sem_act, g - 2)
                    _lab(inst, f"S(g{g},ki{ki})")
                    inst.then_inc(sem_pe)

            def emit_O(p):
                h = p // NPH
                b = h // G
                buf = p % 2
                if p == 0 or p == G * NPH:
                    nc.tensor.wait_ge(sem_v[b], 16)
                    nc.tensor.wait_ge(sem_pool, 5 + b)
                if p >= 2:
                    dve_wait(mults_done[p - 2])   # opsum buf reuse
                vt3 = VT[b][:].rearrange("p (t c) -> p t c", c=132)
                for i, kt in enumerate(O_KTS):
                    half = kt // 8
                    ki = kt % 8
                    inst = nc.tensor.matmul(
                        opsum(buf),
                        PT[p % 3][:, half * 1024 + ki * 128:
                                  half * 1024 + (ki + 1) * 128],
                        vt3[:, kt, 0:129],
                        start=(i == 0), stop=(i == len(O_KTS) - 1),
                        skip_group_check=True,
                    )
                    if i == 0:
                        inst._wait_ge(sem_act, 2 * p + 1)
                    if i == ODD_ACT_I:
                        inst._wait_ge(sem_act, 2 * p + 2)
                    if i == DVE_WAIT_I:
                        inst._wait_ge(sem_dve, dexp_done[2 * p + 1])
                    _lab(inst, f"O(p{p},kt{kt})")
                    inst.then_inc(sem_pe)

            emit_warm(N_WARM)
            emit_ftr(0)           # K b0 t0-7 -> banks 6/7
            emit_warm(N_WARM2)
            emit_ftr(1)           # Q h0 t0-7 -> slot 0
            emit_warm(N_WARM2)
            emit_ftr(2)           # K b0 t8-15 -> slot 1
            emit_S(0)
            emit_S(1)
            for w in range(1, NW):
                if w < NPAIR:
                    emit_S(2 * w)
                if w >= 2:
                    emit_O(w - 2)
                if w < NPAIR:
                    emit_S(2 * w + 1)

        @block.scalar
        def _(sc):
            nc.scalar.wait_ge(sem_pool, 4)
            nc.scalar.activation(                  # preload Exp table
                out=scr[:, 0:1], in_=bias0[:, 0:1],
                func=EXP, bias=bias0[:, 0:1], scale=1.0,
            )
            for g in range(2 * NPAIR):
                p = g >> 1
                half = g & 1
                slot = g % 3
                ov = PT[p % 3][:, :].rearrange("p (s c) -> p s c", c=1024)
                _lab(nc.scalar.activation(
                    out=ov[:, half:half + 1, 0:AW],
                    in_=pv[:, slot:slot + 1, 0:AW],
                    func=EXP, bias=bias0[:, 0:1], scale=SCALE,
                )._wait_ge(sem_pe, pe_after_S[g]),
                    f"exp(g{g})").then_inc(sem_act)

        @block.vector
        def _(ve):
            # fast-path cast copies psum f32 -> KT/QT f16
            for label, pe_val, dst, base in [
                ("cpK0", 8, KT[0][:, 0:1024], TR_BASE[0]),
                ("cpQ0", 16, QT[0][:, 0:1024], TR_BASE[1]),
                ("cpK1", 24, KT[0][:, 1024:2048], TR_BASE[2]),
            ]:
                nc.vector.wait_ge(sem_pe, pe_val)
                _lab(nc.vector.tensor_copy(
                    dst, psum[:, base:base + 1024]), label).then_inc(sem_dve)

            def emit_dexp(g):
                p = g >> 1
                half = g & 1
                slot = g % 3
                nc.vector.wait_ge(sem_pe, pe_after_S[g])
                _lab(nc.vector.tensor_scalar(
                    PTI[p % 3][:, half * 1024 + AW:half * 1024 + 1024],
                    pv[:, slot, AW:1024],
                    A16, B16, op0=mybir.AluOpType.mult,
                    op1=mybir.AluOpType.add,
                ), f"dexp(g{g})").then_inc(sem_dve)

            def emit_recip(p):
                buf = p % 2
                nc.vector.wait_ge(sem_pe, pe_after_O[p])
                if p >= 2:
                    nc.vector.wait_ge(sem_dve, mults_done[p - 2])
                _lab(nc.vector.reciprocal(
                    rsb[buf][:, 0:1], opsum(buf)[:, 128:129]),
                    f"recip({p})").then_inc(sem_dve)

            def emit_mult(p):
                h = p // NPH
                qc = p % NPH
                buf = p % 2
                nc.vector.wait_ge(sem_dve, recips_done[p])
                if qc == 0 and h >= 2:
                    nc.vector.wait_ge(sem_out[h - 2], 64)
                _lab(nc.vector.tensor_scalar(
                    OS[h % 2][:, qc * 128:(qc + 1) * 128],
                    opsum(buf)[:, 0:128],
                    rsb[buf][:, 0:1],
                    None,
                    op0=mybir.AluOpType.mult,
                ), f"mult({p})").then_inc(sem_dve)

            for op in dve_ops:
                if op[0] == "dexp":
                    emit_dexp(op[1])
                elif op[0] == "recip":
                    emit_recip(op[1])
                else:
                    emit_mult(op[1])

    return nc


_NC = None


def _get_nc():
    global _NC
    if _NC is None:
        _NC = build_attention_nc(2048, 2, 4)
    return _NC


def kernel(query, key, value):
    from concourse.bass_utils import run_bass_kernel_spmd

    query = np.ascontiguousarray(query, dtype=np.float32)
    key = np.ascontiguousarray(key, dtype=np.float32)
    value = np.ascontiguousarray(value, dtype=np.float32)
    G = query.shape[2] // key.shape[2]
    nc = _get_nc()
    in_maps = []
    for c in range(N_CORES):
        in_maps.append({
            "query": np.ascontiguousarray(query[:, :, c * G:(c + 1) * G, :]),
            "key": np.ascontiguousarray(key[:, :, c, :]),
            "value": np.ascontiguousarray(value[:, :, c, :]),
        })
    res = run_bass_kernel_spmd(nc, in_maps, list(range(N_CORES)))
    out = np.empty_like(query)
    for c in range(N_CORES):
        out[:, :, c * G:(c + 1) * G, :] = res.results[c]["out"]
    return out
